# revision 1
# baseline (speedup 1.0000x reference)
"""GAT-style attention score kernel for 8 TRN2 NeuronCores.

Computes out[i,j] = LeakyReLU(Wh[i]@a1 + Wh[j]@a2, slope=0.2) for
N=8192, D=64 -> [8192, 8192] f32 output (256MB).

Sharding: output rows across 8 cores ([1024, 8192] slab each). Each core
gets the full transposed Wh (replicated) + its row slice, f16 for the
tiny matmuls; x tiles are bf16; the output is f32. Combined rounding
error ~2e-3 relative-scale.

Per-core pipeline (memory-bound: the 32MB output write is the wall):
  Scalar: issues ALL input DMAs on its own HWDGE queue (the sync queue
          carries nothing but the output stream); copies s1 + the s2
          broadcast quarters PSUM->SBUF; bias-add passes
          x = Identity(s2 + s1[k]) for every piece - tile 0's pieces
          read the PSUM quarters directly so the first output piece
          skips the copy latency.
  PE:     s2 broadcast tile (a2-replicated stationary f16 matmuls),
          s1. Interleaved so the first eighth is ready earliest.
  Vector: out = max(0.2*x, x) via scalar_tensor_tensor (exact
          LeakyReLU; the HW Lrelu table has a hardwired 0.01 slope).
  Sync:   pure output DMA stream; tile 0 leaves as 2 eighths + 3
          quarters, tiles 1-6 as full 4MB tiles, tile 7 as 2 halves
          (tail latency).

Hazard notes (hard-won):
 - Same-engine RAW through SBUF needs a retire guard: wait_ge on the
   producer's own semaphore right after it (the bias read of the first
   x pass raced the 8-element s1 copy's writeback and saw zeros).
 - Every output DMA gets a dedicated semaphore: a shared cumulative
   counter can reach a threshold via mixed per-engine completions of
   different DMAs, unfencing a buffer still being read.
"""

from contextlib import ExitStack

import numpy as np
import concourse.bass as bass
import concourse.mybir as mybir
from concourse.bass_utils import run_bass_kernel_spmd

N = 8192          # nodes
D = 64            # feature dim
M = 8             # cores
ROWS = N // M     # 1024 output rows per core
NT = ROWS // 128  # 8 row tiles of 128 partitions
FCH = 512         # matmul moving-dim chunk
QW = 2048         # quarter width
HW_ = 4096        # half width
NEG_SLOPE = 0.2
N_WARM = 4        # dummy matmuls to ramp the PE clock

# pieces: tile0 = 2 eighths + 3 quarters, tiles 1-6 = halves,
# tile7 = quarters (smaller final DMA => shorter tail) -> 21 pieces
P0 = [(0, 0, 1024), (0, 1024, 2048), (0, 2048, 4096),
      (0, 4096, 6144), (0, 6144, 8192)]
PIECES = list(P0)
for _k in range(1, NT - 1):
    PIECES += [(_k, 0, HW_), (_k, HW_, N)]
PIECES += [(NT - 1, q * QW, (q + 1) * QW) for q in range(4)]

# PSUM source of tile-0 pieces: (psum buffer index, column offset)
#   quarter0 -> ps_a, quarter1 -> ps_b, quarter2 -> ps_a, quarter3 -> ps_b
P0_SRC = [(0, 0), (0, 1024), (1, 0), (0, 0), (1, 0)]
# mm threshold for each tile-0 piece (see PE program numbering below)
P0_MM = [2, 12, 16, 20, 24]

_cache = {}


def _build():
    nc = bass.Bass()
    f32 = mybir.dt.float32
    f16 = mybir.dt.float16
    bf16 = mybir.dt.bfloat16

    whT_ext = nc.declare_dram_parameter("whT", [D, N], f16, isOutput=False)
    whTr_ext = nc.declare_dram_parameter("whTr", [D, ROWS], f16, isOutput=False)
    a1_ext = nc.declare_dram_parameter("a1", [D, 1], f16, isOutput=False)
    a2r_ext = nc.declare_dram_parameter("a2r", [D, 128], f16, isOutput=False)
    out_ext = nc.declare_dram_parameter("out", [ROWS, N], f32, isOutput=True)

    with ExitStack() as ctx:
        sb_whT = ctx.enter_context(nc.sbuf_tensor("sb_whT", [D, N], f16))
        sb_whTr = ctx.enter_context(nc.sbuf_tensor("sb_whTr", [D, ROWS], f16))
        sb_a1 = ctx.enter_context(nc.sbuf_tensor("sb_a1", [D, 1], f16))
        sb_a2r = ctx.enter_context(nc.sbuf_tensor("sb_a2r", [D, 128], f16))
        sb_s1 = ctx.enter_context(nc.sbuf_tensor("sb_s1", [128, NT], f32))
        sb_s2b = ctx.enter_context(nc.sbuf_tensor("sb_s2b", [128, N], f32))
        sb_x0 = ctx.enter_context(nc.sbuf_tensor("sb_x0", [128, HW_], bf16))
        sb_x1 = ctx.enter_context(nc.sbuf_tensor("sb_x1", [128, HW_], bf16))
        sb_o0 = ctx.enter_context(nc.sbuf_tensor("sb_o0", [128, N], f32))
        sb_o1 = ctx.enter_context(nc.sbuf_tensor("sb_o1", [128, N], f32))
        sb_o2 = ctx.enter_context(nc.sbuf_tensor("sb_o2", [128, N], f32))
        sb_junk = ctx.enter_context(nc.sbuf_tensor("sb_junk", [128, 1], f32))
        ps_a = ctx.enter_context(nc.psum_tensor("ps_a", [128, QW], f32))
        ps_b = ctx.enter_context(nc.psum_tensor("ps_b", [128, QW], f32))
        din = ctx.enter_context(nc.semaphore("din"))
        dwh = [ctx.enter_context(nc.semaphore(f"dwh{c}")) for c in range(4)]
        mm = ctx.enter_context(nc.semaphore("mm"))
        scp = ctx.enter_context(nc.semaphore("scp"))
        cq = ctx.enter_context(nc.semaphore("cq"))
        xs = ctx.enter_context(nc.semaphore("xs"))
        sst = ctx.enter_context(nc.semaphore("sst"))
        q0d = ctx.enter_context(nc.semaphore("q0d"))          # tile-0 piece DMAs
        tkd = [ctx.enter_context(nc.semaphore(f"t{k}d")) for k in range(1, NT)]
        block = ctx.enter_context(nc.Block())
        sb_x = [sb_x0, sb_x1]
        sb_o = [sb_o0, sb_o1, sb_o2]
        ps = [ps_a, ps_b]

        @block.sync
        def _(sync):
            # pure output stream; tile 0 leaves in 5 pieces
            for px, (k, lo, hi) in enumerate(P0):
                sync.wait_ge(sst, px + 1)
                sync.dma_start(
                    out_ext[0:128, lo:hi], sb_o0[:, lo:hi]
                ).then_inc(q0d, 16)
            for k in range(1, NT - 1):
                sync.wait_ge(sst, 2 * k + 5)  # both halves of tile k done
                sync.dma_start(
                    out_ext[k * 128:(k + 1) * 128, :], sb_o[k % 3][:, :]
                ).then_inc(tkd[k - 1], 16)
            # tile 7 leaves in quarters to shave tail latency
            k = NT - 1
            for h in range(4):
                sync.wait_ge(sst, 18 + h)
                sync.dma_start(
                    out_ext[k * 128:(k + 1) * 128, h * QW:(h + 1) * QW],
                    sb_o[k % 3][:, h * QW:(h + 1) * QW],
                ).then_inc(tkd[k - 1], 16)

        @block.tensor
        def _(tensor):
            # ramp the PE clock on garbage while input DMAs fly
            for w in range(N_WARM):
                tensor.matmul(
                    ps_b[:, (w % 4) * FCH:(w % 4 + 1) * FCH],
                    sb_whTr[:, 0:128],
                    sb_whT[:, 0:FCH],
                )
            # first eighth of s2b quarter 0: mm 1-2
            tensor.wait_ge(din, 16)
            tensor.wait_ge(dwh[0], 16)
            for j in range(2):
                tensor.matmul(
                    ps_a[:, j * FCH:(j + 1) * FCH],
                    sb_a2r[:, :],
                    sb_whT[:, j * FCH:(j + 1) * FCH],
                ).then_inc(mm)
            # s1 into ps_b cols 1024..1031: k0 first (mm 3) - it alone
            # gates tile 0's bias - then k1-7 (mm 4-10)
            tensor.wait_ge(din, 48)
            for k in range(NT):
                tensor.matmul(
                    ps_b[:, 1024 + k:1024 + k + 1],
                    sb_whTr[:, k * 128:(k + 1) * 128],
                    sb_a1[:, :],
                ).then_inc(mm)
            # rest of quarter 0: mm 11-12
            for j in range(2, 4):
                tensor.matmul(
                    ps_a[:, j * FCH:(j + 1) * FCH],
                    sb_a2r[:, :],
                    sb_whT[:, j * FCH:(j + 1) * FCH],
                ).then_inc(mm)
            # s2b quarters 1-3: mm 13-24 (psum b, a, b)
            for qq in range(1, 4):
                tensor.wait_ge(dwh[qq], 16)
                if qq == 1:
                    tensor.wait_ge(scp, 2)      # s1 fully evacuated from ps_b
                else:
                    tensor.wait_ge(cq, qq - 1)  # psum buf drained (also fences
                    #                             tile-0 x reads: they precede
                    #                             the copy on the same engine)
                for j in range(4 * qq, 4 * qq + 4):
                    tensor.matmul(
                        ps[qq % 2][:, (j % 4) * FCH:(j % 4 + 1) * FCH],
                        sb_a2r[:, :],
                        sb_whT[:, j * FCH:(j + 1) * FCH],
                    ).then_inc(mm)

        @block.scalar
        def _(scalar):
            # all input DMAs ride the scalar HWDGE queue, away from output
            scalar.dma_start(sb_whT[:, 0:QW], whT_ext[:, 0:QW]).then_inc(dwh[0], 16)
            scalar.dma_start(sb_a2r[:, :], a2r_ext[:, :]).then_inc(din, 16)
            scalar.dma_start(sb_a1[:, :], a1_ext[:, :]).then_inc(din, 16)
            scalar.dma_start(sb_whTr[:, :], whTr_ext[:, :]).then_inc(din, 16)
            for c in range(1, 4):
                scalar.dma_start(
                    sb_whT[:, c * QW:(c + 1) * QW], whT_ext[:, c * QW:(c + 1) * QW]
                ).then_inc(dwh[c], 16)
            # warm the activation table while they fly
            scalar.activation(
                sb_junk[:, :], sb_junk[:, :],
                mybir.ActivationFunctionType.Identity,
                bias=sb_junk[:, 0:1], scale=1.0,
            )
            # s1 copies first: col 0 gates tile-0 bias, cols 1-7 unblock
            # PE quarter 1 (scp>=2); retire-guarded before any bias read
            scalar.wait_ge(mm, 3)
            scalar.copy(sb_s1[:, 0:1], ps_b[:, 1024:1025]).then_inc(scp)
            scalar.wait_ge(mm, 10)
            scalar.copy(sb_s1[:, 1:NT], ps_b[:, 1025:1024 + NT]).then_inc(scp)
            scalar.wait_ge(scp, 2)   # RAW guard: s1 visible
            for px, (k, lo, hi) in enumerate(PIECES):
                if k == 0:
                    # x straight from the PSUM quarter; copy to s2b after
                    pb, po = P0_SRC[px]
                    scalar.wait_ge(mm, P0_MM[px])
                    if px >= 2:
                        scalar.wait_ge(sst, px - 1)  # x buf px%2 consumed
                    scalar.activation(
                        sb_x[px % 2][:, 0:hi - lo],
                        ps[pb][:, po:po + hi - lo],
                        mybir.ActivationFunctionType.Identity,
                        bias=sb_s1[:, 0:1], scale=1.0,
                    ).then_inc(xs)
                    # trailing copies keep s2b for tiles 1-7 and free PSUM
                    if px == 1:
                        scalar.copy(sb_s2b[:, 0:QW], ps_a[:, :]).then_inc(cq)
                    elif px >= 2:
                        q = px - 1
                        scalar.copy(
                            sb_s2b[:, q * QW:(q + 1) * QW], ps[q % 2][:, :]
                        ).then_inc(cq)
                else:
                    if px == 5:
                        scalar.wait_ge(cq, 2)   # s2b halves 0-1 resident
                        scalar.wait_ge(scp, 2)  # s1 cols 1-7 retired
                    elif px == 6:
                        scalar.wait_ge(cq, 4)   # full s2b resident
                    scalar.wait_ge(sst, px - 1)  # x buf px%2 consumed
                    scalar.activation(
                        sb_x[px % 2][:, 0:hi - lo],
                        sb_s2b[:, lo:hi],
                        mybir.ActivationFunctionType.Identity,
                        bias=sb_s1[:, k:k + 1], scale=1.0,
                    ).then_inc(xs)

        @block.vector
        def _(vector):
            for px, (k, lo, hi) in enumerate(PIECES):
                vector.wait_ge(xs, px + 1)
                if lo == 0 and k >= 3:
                    # out buf k%3 free: tile k-3's own DMA fully complete
                    if k == 3:
                        vector.wait_ge(q0d, 80)  # all 5 tile-0 piece DMAs
                    else:
                        vector.wait_ge(tkd[k - 4], 16)
                vector.scalar_tensor_tensor(
                    sb_o[k % 3][:, lo:hi],
                    sb_x[px % 2][:, 0:hi - lo],
                    NEG_SLOPE,
                    sb_x[px % 2][:, 0:hi - lo],
                    mybir.AluOpType.mult,
                    mybir.AluOpType.max,
                ).then_inc(sst)

    return nc


def _run(Wh, a, trace=False, **kw):
    Wh = np.ascontiguousarray(np.asarray(Wh, dtype=np.float32))
    a = np.ascontiguousarray(np.asarray(a, dtype=np.float32))
    assert Wh.shape == (N, D) and a.shape == (2 * D, 1)

    if "nc" not in _cache:
        _cache["nc"] = _build()
    nc = _cache["nc"]

    WhT16 = np.ascontiguousarray(Wh.T.astype(np.float16))        # [64, 8192]
    a1 = np.ascontiguousarray(a[:D, :].astype(np.float16))       # [64, 1]
    a2r = np.ascontiguousarray(np.tile(a[D:, :].astype(np.float16), (1, 128)))
    in_maps = [
        {
            "whT": WhT16,
            "whTr": np.ascontiguousarray(WhT16[:, i * ROWS:(i + 1) * ROWS]),
            "a1": a1,
            "a2r": a2r,
        }
        for i in range(M)
    ]
    res = run_bass_kernel_spmd(nc, in_maps, core_ids=list(range(M)), trace=trace, **kw)
    out = np.concatenate([res.results[i]["out"] for i in range(M)], axis=0)
    return out, res


def kernel(Wh, a):
    return _run(Wh, a)[0]

